# revision 4
# baseline (speedup 1.0000x reference)
"""Causal multi-head attention block (qkv proj + causal softmax attention + out proj)
for Trainium2, sharded over 8 NeuronCores: data-parallel over batch (2) x
tensor-parallel over heads (4 heads per core of 16).

Each core computes, for its batch b and its 4 heads:
  qT,kT [hd, S] (head-pair tiles: partitions hh*64 hold head 2p+hh)
  ST    [k, q] scores, row-tiled 64x128 PE mode: both heads of a pair
        computed concurrently in the systolic array
  P     = exp(ST - shift)  (e4m3 for q-chunks 1-3, bf16 for chunk 0)
  attnT [80, q]: fp8 DoubleRow matmul processes two key tiles per
        instruction: [v_kt | v_kt+1] packed [128, 2, 80] e4m3, with the
        ones column at col 64 producing the softmax denominator in row 64
  attn_n = attnT / denom
  out_partial [S, D] = attn_n.T @ owT  (row-parallel out proj)
Host sums the 4 per-core partials of each batch.
"""

import os
import sys

import numpy as np

sys.path.insert(0, "/opt/trn_rl_repo")

import concourse.bass as bass
import concourse.tile as tile
from concourse import bacc, mybir
from concourse.bass import MemorySpace
from concourse.bass_utils import run_bass_kernel_spmd

F32 = mybir.dt.float32
F8 = mybir.dt.float8e4
BF16 = mybir.dt.bfloat16
EXP = mybir.ActivationFunctionType.Exp
DR = mybir.MatmulPerfMode.DoubleRow

B, S, D = 2, 2048, 1024
H, HD = 16, 64
NCORES = 8
NH = 4          # heads per core
NP = 2          # head pairs per core
SCALE = HD ** -0.5
SHIFT = 2.0     # P = exp(s - SHIFT) on fp8 chunks; cancels in num/den

N_DT = D // 128          # 8 d-tiles of 128
N_ST = S // 128          # 16 seq tiles of 128
N_CH = S // 512          # 4 seq chunks of 512
FQK = 4                  # f-tiles covering q|k (pair-major)
VW = NH * HD             # 256 v columns
VP = 80                  # padded attnT rows (65 used; 80 for fp8 DR step%16)
NEG = -1.0e9

MM_DT = BF16


def _emit(tc, nc, xT_d, wT_d, owT_d, mask_d, out_d):
    import contextlib

    ctx = contextlib.ExitStack()
    with ctx:
        # ---- pools (PSUM: 2 acc + 2 at + 2x2 stp = 8 banks) ----
        sb = ctx.enter_context(tc.tile_pool(name="sb", bufs=1))
        p8_pool = ctx.enter_context(tc.tile_pool(name="p8p", bufs=4))
        an_pool = ctx.enter_context(tc.tile_pool(name="attn_n", bufs=4))
        sm_pool = ctx.enter_context(tc.tile_pool(name="smalls", bufs=8))
        out_pool = ctx.enter_context(tc.tile_pool(name="outsb", bufs=4))
        ps_mm = ctx.enter_context(
            tc.tile_pool(name="ps_mm", bufs=2, space=MemorySpace.PSUM))
        ps_at = ctx.enter_context(
            tc.tile_pool(name="ps_at", bufs=2, space=MemorySpace.PSUM))
        ps_st = ctx.enter_context(
            tc.tile_pool(name="ps_st", bufs=2, space=MemorySpace.PSUM))

        # qT/kT head-pair tiles: f 0,1 = q pairs 0,1; f 2,3 = k pairs.
        # partitions hh*64:(hh+1)*64 hold head 2p+hh.
        qk_sb = [sb.tile([128, S], MM_DT, tag=f"qk{i}", name=f"qk{i}")
                 for i in range(4)]
        qp = qk_sb[0:2]
        kp = qk_sb[2:4]
        # HAM warm-up: dependency-free matmuls run while the input DMAs
        # stream, so the PE clock-gate is already 8/8 when real work starts.
        warm_sb = sb.tile([128, 512], MM_DT)
        nc.vector.memset(warm_sb, 0.0)
        wu_ps = ps_st.tile([128, 2, 512], F32, tag="stp", name="wu_ps")
        for _ in range(30):
            nc.tensor.matmul(wu_ps[:, 0, :], warm_sb[:, 0:128], warm_sb,
                             start=True, stop=True)
        # v fp8 pairs: v8[ktp][:, par, h, 0:64] = v of seq tile 2*ktp+par,
        # col 64 = ones (denominator row), cols 65:80 zero padding.
        v8 = [sb.tile([128, 2, NH, VP], F8, tag=f"v8_{i}", name=f"v8_{i}")
              for i in range(N_ST // 2)]
        for t in v8:
            nc.gpsimd.memset(t, 0.0)
            nc.vector.memset(t[:, :, :, HD:HD + 1], 1.0)
        # v bf16 tiles for chunk 0 (kt 0..3)
        v16 = [sb.tile([128, NH, HD + 1], MM_DT, tag=f"v16_{i}", name=f"v16_{i}")
               for i in range(4)]
        ones_sb = sb.tile([128, NH], F32)
        nc.vector.memset(ones_sb, 1.0)
        shift_sb = sb.tile([128, 1], F32)
        nc.vector.memset(shift_sb, -SHIFT)
        mask_sb = sb.tile([128, 640], F32)
        owT_sb = [sb.tile([128, D], MM_DT, tag=f"ow{i}", name=f"ow{i}")
                  for i in range(NP)]
        xT_sb = [sb.tile([128, S], MM_DT, tag=f"x{i}", name=f"x{i}")
                 for i in range(N_DT)]
        wT_sb = [sb.tile([128, 3 * VW], MM_DT, tag=f"w{i}", name=f"w{i}")
                 for i in range(N_DT)]

        nc.sync.dma_start(out=mask_sb, in_=mask_d)
        for p in range(NP):
            nc.sync.dma_start(out=owT_sb[p], in_=owT_d[p * 128:(p + 1) * 128, :])
        for i in range(N_DT):
            nc.sync.dma_start(out=xT_sb[i], in_=xT_d[i * 128:(i + 1) * 128, :])
            nc.sync.dma_start(out=wT_sb[i], in_=wT_d[i * 128:(i + 1) * 128, :])

        # ---------------- phase 1: qkv projection ----------------
        # qT/kT: out[f 128, s 512] += wT[d, f].T @ xT[d, s]; f-tile = head pair
        for f in range(FQK):
            for sch in range(N_CH):
                pss = ps_mm.tile([128, 512], F32, tag="acc", name="psqk")
                for d in range(N_DT):
                    nc.tensor.matmul(
                        pss,
                        wT_sb[d][:, f * 128:(f + 1) * 128],
                        xT_sb[d][:, sch * 512:(sch + 1) * 512],
                        start=(d == 0),
                        stop=(d == N_DT - 1),
                    )
                nc.scalar.copy(qk_sb[f][:, sch * 512:(sch + 1) * 512], pss)

        # v: out[s 128, 256] += xT[d, s].T @ wvT[d, 256]
        for st in range(N_ST):
            psvt = ps_mm.tile([128, VW], F32, tag="acc", name="psv")
            for d in range(N_DT):
                nc.tensor.matmul(
                    psvt,
                    xT_sb[d][:, st * 128:(st + 1) * 128],
                    wT_sb[d][:, 2 * VW:3 * VW],
                    start=(d == 0),
                    stop=(d == N_DT - 1),
                )
            ktp, par = divmod(st, 2)
            nc.scalar.copy(
                v8[ktp][:, par, :, 0:HD],
                psvt.rearrange("p (h d) -> p h d", h=NH),
            )
            if st < 4:
                nc.scalar.copy(
                    v16[st][:, :, 0:HD],
                    psvt.rearrange("p (h d) -> p h d", h=NH),
                )
                nc.vector.tensor_copy(v16[st][:, :, HD:HD + 1], ones_sb)

        # ---------------- phase 2+3: attention + out projection ----------------
        # Emission is software-pipelined: chunk qc+1's attention is emitted
        # before chunk qc's out-projection so the in-order PE queue never
        # stalls on the (DVE+GpSimd) softmax-denominator normalize chain.
        an_hist = {}
        pend_norm = {}

        def emit_attention(qc):
            fp8 = qc > 0
            n_ktp = 2 * (qc + 1)
            an = [an_pool.tile([128, 512], MM_DT, tag=f"an{p}", name=f"an{p}")
                  for p in range(NP)]
            an_hist[qc] = an
            for p in range(NP):
                at_ps = [ps_at.tile([VP, 512], F32, tag="at", name="at_ps")
                         for _ in range(2)]
                # one-step software pipeline: scores/exp for pair ktp overlap
                # the attnT accumulation of ktp-1, so the in-order PE never
                # waits on the ACT exp chain.
                pend = {}
                DEPTH = 1
                for ktp in range(n_ktp + DEPTH):
                    if ktp < n_ktp:
                        kt0 = 2 * ktp
                        jA = kt0 - 4 * qc
                        rs = 0 if jA < 0 else jA * 128  # pair-shared col start
                        if fp8:
                            pt = p8_pool.tile([128, 2, 2, 512], F8,
                                              tag="p8", name="p8")
                            if jA >= 0:
                                # kt0+1 contributes nothing on cols
                                # [rs:rs+128); zero its P block there
                                nc.gpsimd.memset(pt[:, 1, :, rs:rs + 128], 0.0)
                        else:
                            pt = p8_pool.tile([128, 2, 2, 512], MM_DT,
                                              tag="p16", name="p16", bufs=2)
                        for par in range(2):
                            kt = kt0 + par
                            j = kt - 4 * qc
                            rss = 0 if j < 0 else j * 128
                            stp = ps_st.tile([128, 2, 512], F32,
                                             tag="stp", name="stp")
                            for hh in range(2):
                                nc.tensor.matmul(
                                    stp[:, hh, rss:512],
                                    kp[p][hh * HD:(hh + 1) * HD,
                                          kt * 128:(kt + 1) * 128],
                                    qp[p][hh * HD:(hh + 1) * HD,
                                          qc * 512 + rss:(qc + 1) * 512],
                                    start=True,
                                    stop=True,
                                )
                            if j >= 0:
                                # additive triangular mask on the 128 cols
                                # that cross the diagonal
                                for hh in range(2):
                                    nc.vector.tensor_add(
                                        stp[:, hh, rss:rss + 128],
                                        stp[:, hh, rss:rss + 128],
                                        mask_sb[:, 128:256],
                                    )
                            # one EXP covers both heads of this key tile
                            nc.scalar.activation(
                                pt[:, par, :, rss:512],
                                stp[:, :, rss:512],
                                EXP,
                                bias=(shift_sb if fp8 else 0.0),
                            )
                        pend[ktp] = (pt, rs)
                    if ktp >= DEPTH:
                        ktp2 = ktp - DEPTH
                        pt, rs2 = pend.pop(ktp2)
                        if fp8:
                            for hh in range(2):
                                nc.tensor.matmul(
                                    at_ps[hh][:, rs2:512],
                                    v8[ktp2][:, :, 2 * p + hh, :],
                                    pt[:, :, hh, rs2:512],
                                    start=(ktp2 == 0),
                                    stop=(ktp2 == n_ktp - 1),
                                    perf_mode=DR,
                                )
                        else:
                            for par in range(2):
                                kt = 2 * ktp2 + par
                                j = kt - 4 * qc
                                rss = 0 if j < 0 else j * 128
                                for hh in range(2):
                                    nc.tensor.matmul(
                                        at_ps[hh][0:HD + 1, rss:512],
                                        v16[kt][:, 2 * p + hh, :],
                                        pt[:, par, hh, rss:512],
                                        start=(kt == 0),
                                        stop=(kt == 2 * n_ktp - 1),
                                    )
                # Release the accumulator banks fast (two DVE copies);
                # the slow reciprocal chain is deferred to emit_normalize
                # so it never sits ahead of the next segment's DVE work.
                for hh in range(2):
                    anu = an_pool.tile([HD, 512], MM_DT, tag="anu", name="anu")
                    nc.vector.tensor_copy(anu, at_ps[hh][0:HD, :])
                    lsb = sm_pool.tile([1, 512], F32, tag="lsb", name="lsb")
                    nc.vector.tensor_copy(lsb, at_ps[hh][HD:HD + 1, :])
                    pend_norm[(qc, p, hh)] = (anu, lsb)

        def emit_normalize(qc, pairs=range(NP)):
            an = an_hist[qc]
            for p in pairs:
                for hh in range(2):
                    anu, lsb = pend_norm.pop((qc, p, hh))
                    rec = sm_pool.tile([1, 512], F32, tag="rec", name="rec")
                    nc.vector.reciprocal_approx_fast(rec, lsb)
                    bc = sm_pool.tile([HD, 512], F32, tag="bc", name="bc")
                    nc.gpsimd.partition_broadcast(bc, rec)
                    nc.vector.tensor_mul(
                        an[p][hh * HD:(hh + 1) * HD, :], anu, bc,
                    )

        def emit_outproj(qc):
            an = an_hist.pop(qc)
            for qs in range(4):
                qsl = slice(qs * 128, (qs + 1) * 128)
                for e in range(2):
                    ops = ps_mm.tile([128, 512], F32, tag="acc", name="psout")
                    for p in range(NP):
                        nc.tensor.matmul(
                            ops,
                            an[p][:, qsl],
                            owT_sb[p][:, e * 512:(e + 1) * 512],
                            start=(p == 0),
                            stop=(p == NP - 1),
                        )
                    osb = out_pool.tile([128, 512], F32, tag="osb", name="osb")
                    nc.vector.tensor_copy(osb, ops)
                    nc.sync.dma_start(
                        out=out_d[qc * 512 + qs * 128:qc * 512 + (qs + 1) * 128,
                                  e * 512:(e + 1) * 512],
                        in_=osb,
                    )

        order = [0, 1, 2, 3]
        prev = None
        for qc in order:
            emit_attention(qc)
            if prev is not None:
                emit_normalize(prev)
                emit_outproj(prev)
            prev = qc
        emit_normalize(prev)
        emit_outproj(prev)


_CACHE = {}


def _build():
    if "nc" in _CACHE:
        return _CACHE["nc"]
    nc = bacc.Bacc("TRN2", target_bir_lowering=False, debug=False)
    xT_d = nc.dram_tensor("xT", [D, S], MM_DT, kind="ExternalInput").ap()
    wT_d = nc.dram_tensor("wT", [D, 3 * VW], MM_DT, kind="ExternalInput").ap()
    owT_d = nc.dram_tensor("owT", [VW, D], MM_DT, kind="ExternalInput").ap()
    mask_d = nc.dram_tensor("mask", [128, 640], F32, kind="ExternalInput").ap()
    out_d = nc.dram_tensor("out", [S, D], F32, kind="ExternalOutput").ap()
    with tile.TileContext(nc) as tc:
        _emit(tc, nc, xT_d, wT_d, owT_d, mask_d, out_d)
    nc.compile()
    _CACHE["nc"] = nc
    return nc


def _mask_np():
    # [128, 640]: cols 0-127 all NEG, cols 128-255 lower-triangular keep
    # (col >= row -> 0 else NEG), cols 256-639 zeros.
    m = np.zeros((128, 640), np.float32)
    m[:, 0:128] = NEG
    r = np.arange(128)
    tri = np.where(r[None, :] >= r[:, None], 0.0, NEG).astype(np.float32)
    m[:, 128:256] = tri
    return m


def make_in_maps(x, qkv_w, out_w):
    """Per-core input dicts for the 8-way (batch x head-group) sharding."""
    x = np.asarray(x, np.float32)
    qkv_w = np.asarray(qkv_w, np.float32)
    out_w = np.asarray(out_w, np.float32)
    xT = [np.ascontiguousarray(x[b].T) for b in range(B)]
    mask = _mask_np()
    import ml_dtypes
    np_mm = ml_dtypes.bfloat16
    in_maps = []
    for c in range(NCORES):
        b = c // 4
        h0 = (c % 4) * NH
        rows = np.arange(h0 * HD, (h0 + NH) * HD)
        wq = qkv_w[rows] * np.float32(SCALE)
        wk = qkv_w[D + rows]
        wv = qkv_w[2 * D + rows]
        wT = np.ascontiguousarray(np.concatenate([wq, wk, wv], 0).T)
        owT = np.ascontiguousarray(out_w[:, rows].T)
        in_maps.append({"xT": xT[b].astype(np_mm), "wT": wT.astype(np_mm),
                        "owT": owT.astype(np_mm), "mask": mask})
    return in_maps


def kernel(x, qkv_w, out_w, _trace=False, _trace_cores=None):
    nc = _build()
    in_maps = make_in_maps(x, qkv_w, out_w)
    res = run_bass_kernel_spmd(
        nc, in_maps, core_ids=list(range(NCORES)),
        trace=_trace, trace_cores=_trace_cores,
    )
    outs = [r["out"] for r in res.results]
    full = np.stack([
        outs[0] + outs[1] + outs[2] + outs[3],
        outs[4] + outs[5] + outs[6] + outs[7],
    ]).astype(np.float32)
    if _trace:
        return full, res
    return full


# revision 5
# speedup vs baseline: 1.1056x; 1.1056x over previous
"""Causal multi-head attention block (qkv proj + causal softmax attention + out proj)
for Trainium2, sharded over 8 NeuronCores: data-parallel over batch (2) x
tensor-parallel over heads (4 heads per core of 16).

Each core computes, for its batch b and its 4 heads:
  qT,kT [hd, S] (head-pair tiles: partitions hh*64 hold head 2p+hh)
  ST    [k, q] scores, row-tiled 64x128 PE mode: both heads of a pair
        computed concurrently in the systolic array (explicit tile_position)
  P     = exp(ST - shift)  (e4m3 for q-chunks 1-3, bf16 for chunk 0)
  attnT [80, q]: fp8 DoubleRow matmul processes two key tiles per
        instruction: [v_kt | v_kt+1] packed [128, 2, 80] e4m3, with the
        ones column at col 64 producing the softmax denominator in row 64
  attn_n = attnT / denom
  out_partial [S, D] = attn_n.T @ owT  (row-parallel out proj, bf16 partials)
Host sums the 4 per-core partials of each batch in f32.

The qkv projection is not a separate phase: only the minimum prefix for
q-chunk 0 runs up front; the remaining projection groups are interleaved
as fillers between attention steps so the PE stays busy while the ACT
engine (exp, the critical resource of the attention phase) streams.
Chunk order 0,2,3,1 keeps every chunk's prerequisites ready just in time.
"""

import os
import sys

import numpy as np

sys.path.insert(0, "/opt/trn_rl_repo")

import concourse.bass as bass
import concourse.tile as tile
from concourse import bacc, mybir
from concourse.bass import MemorySpace
from concourse.bass_utils import run_bass_kernel_spmd

F32 = mybir.dt.float32
F8 = mybir.dt.float8e4
BF16 = mybir.dt.bfloat16
EXP = mybir.ActivationFunctionType.Exp
DR = mybir.MatmulPerfMode.DoubleRow

B, S, D = 2, 2048, 1024
H, HD = 16, 64
NCORES = 8
NH = 4          # heads per core
NP = 2          # head pairs per core
SCALE = HD ** -0.5
SHIFT = 2.0     # P = exp(s - SHIFT) on fp8 chunks; cancels in num/den

N_DT = D // 128          # 8 d-tiles of 128
N_ST = S // 128          # 16 seq tiles of 128
N_CH = S // 512          # 4 seq chunks of 512
VW = NH * HD             # 256 v columns
VP = 80                  # padded attnT rows (65 used; 80 for fp8 DR step%16)
NEG = -1.0e9

MM_DT = BF16


def _emit(tc, nc, xT_d, wT_d, owT_d, mask_d, out_d):
    import contextlib

    ctx = contextlib.ExitStack()
    with ctx:
        # ---- pools (PSUM: 2 acc + 2 at + 2x2 stp = 8 banks) ----
        sb = ctx.enter_context(tc.tile_pool(name="sb", bufs=1))
        p8_pool = ctx.enter_context(tc.tile_pool(name="p8p", bufs=4))
        an_pool = ctx.enter_context(tc.tile_pool(name="attn_n", bufs=4))
        sm_pool = ctx.enter_context(tc.tile_pool(name="smalls", bufs=8))
        out_pool = ctx.enter_context(tc.tile_pool(name="outsb", bufs=4))
        ps_mm = ctx.enter_context(
            tc.tile_pool(name="ps_mm", bufs=2, space=MemorySpace.PSUM))
        ps_at = ctx.enter_context(
            tc.tile_pool(name="ps_at", bufs=2, space=MemorySpace.PSUM))
        ps_st = ctx.enter_context(
            tc.tile_pool(name="ps_st", bufs=2, space=MemorySpace.PSUM))

        # qT/kT head-pair tiles: f 0,1 = q pairs 0,1; f 2,3 = k pairs.
        # partitions hh*64:(hh+1)*64 hold head 2p+hh.
        qk_sb = [sb.tile([128, S], MM_DT, tag=f"qk{i}", name=f"qk{i}")
                 for i in range(4)]
        qp = qk_sb[0:2]
        kp = qk_sb[2:4]
        # HAM warm-up: dependency-free matmuls run while the input DMAs
        # stream, so the PE clock-gate is already 8/8 when real work starts.
        warm_sb = sb.tile([128, 512], MM_DT)
        nc.vector.memset(warm_sb, 0.0)
        wu_ps = ps_st.tile([128, 2, 512], F32, tag="stp", name="wu_ps")
        for _ in range(30):
            nc.tensor.matmul(wu_ps[:, 0, :], warm_sb[:, 0:128], warm_sb,
                             start=True, stop=True)
        # v fp8 pairs: v8[ktp][:, par, h, 0:64] = v of seq tile 2*ktp+par,
        # col 64 = ones (denominator row), cols 65:80 zero padding.
        v8 = [sb.tile([128, 2, NH, VP], F8, tag=f"v8_{i}", name=f"v8_{i}")
              for i in range(N_ST // 2)]
        for t in v8:
            nc.gpsimd.memset(t, 0.0)
            nc.vector.memset(t[:, :, :, HD:HD + 1], 1.0)
        # v bf16 tiles for chunk 0 (kt 0..3)
        v16 = [sb.tile([128, NH, HD + 1], MM_DT, tag=f"v16_{i}", name=f"v16_{i}")
               for i in range(4)]
        ones_sb = sb.tile([128, NH], F32)
        nc.vector.memset(ones_sb, 1.0)
        shift_sb = sb.tile([128, 1], F32)
        nc.vector.memset(shift_sb, -SHIFT)
        mask_sb = sb.tile([128, 640], F32)
        owT_sb = [sb.tile([128, D], MM_DT, tag=f"ow{i}", name=f"ow{i}")
                  for i in range(NP)]
        xT_sb = [sb.tile([128, S], MM_DT, tag=f"x{i}", name=f"x{i}")
                 for i in range(N_DT)]
        wT_sb = [sb.tile([128, 3 * VW], MM_DT, tag=f"w{i}", name=f"w{i}")
                 for i in range(N_DT)]

        nc.sync.dma_start(out=mask_sb, in_=mask_d)
        for p in range(NP):
            nc.sync.dma_start(out=owT_sb[p], in_=owT_d[p * 128:(p + 1) * 128, :])
        for i in range(N_DT):
            nc.sync.dma_start(out=xT_sb[i], in_=xT_d[i * 128:(i + 1) * 128, :])
            nc.sync.dma_start(out=wT_sb[i], in_=wT_d[i * 128:(i + 1) * 128, :])

        # ---------------- projection group emitters ----------------
        def qk_group(f, sch):
            def run():
                pss = ps_mm.tile([128, 512], F32, tag="acc", name="psqk")
                for d in range(N_DT):
                    nc.tensor.matmul(
                        pss,
                        wT_sb[d][:, f * 128:(f + 1) * 128],
                        xT_sb[d][:, sch * 512:(sch + 1) * 512],
                        start=(d == 0),
                        stop=(d == N_DT - 1),
                    )
                nc.scalar.copy(qk_sb[f][:, sch * 512:(sch + 1) * 512], pss)
            return run

        def v_group(st):
            def run():
                psvt = ps_mm.tile([128, VW], F32, tag="acc", name="psv")
                for d in range(N_DT):
                    nc.tensor.matmul(
                        psvt,
                        xT_sb[d][:, st * 128:(st + 1) * 128],
                        wT_sb[d][:, 2 * VW:3 * VW],
                        start=(d == 0),
                        stop=(d == N_DT - 1),
                    )
                ktp, par = divmod(st, 2)
                nc.scalar.copy(
                    v8[ktp][:, par, :, 0:HD],
                    psvt.rearrange("p (h d) -> p h d", h=NH),
                )
                if st < 4:
                    nc.scalar.copy(
                        v16[st][:, :, 0:HD],
                        psvt.rearrange("p (h d) -> p h d", h=NH),
                    )
                    nc.vector.tensor_copy(v16[st][:, :, HD:HD + 1], ones_sb)
            return run

        # prefix: exactly what q-chunk 0's attention needs
        for g in [qk_group(2, 0), qk_group(3, 0),
                  v_group(0), v_group(1), v_group(2), v_group(3),
                  qk_group(0, 0), qk_group(1, 0)]:
            g()

        # ---------------- attention + out projection ----------------
        an_hist = {}
        pend_norm = {}

        def emit_attention(qc, fillers, last=False):
            fillers = list(fillers)
            fp8 = qc > 0
            n_ktp = 2 * (qc + 1)
            slots = NP * (n_ktp + 1)
            an = [an_pool.tile([128, 512], MM_DT, tag=f"an{p}", name=f"an{p}")
                  for p in range(NP)]
            an_hist[qc] = an

            def fill():
                nonlocal slots
                k = -(-len(fillers) // slots) if slots > 0 else len(fillers)
                for _ in range(k):
                    if fillers:
                        fillers.pop(0)()
                slots -= 1

            for p in range(NP):
                at_ps = [ps_at.tile([VP, 512], F32, tag="at", name="at_ps")
                         for _ in range(2)]
                # one-step software pipeline: scores/exp for pair ktp overlap
                # the attnT accumulation of ktp-1, so the in-order PE never
                # waits on the ACT exp chain.
                pend = {}
                DEPTH = 1
                for ktp in range(n_ktp + DEPTH):
                    if ktp < n_ktp:
                        kt0 = 2 * ktp
                        jA = kt0 - 4 * qc
                        rs = 0 if jA < 0 else jA * 128  # pair-shared col start
                        if fp8:
                            pt = p8_pool.tile([128, 2, 2, 512], F8,
                                              tag="p8", name="p8")
                            if jA >= 0:
                                # kt0+1 contributes nothing on cols
                                # [rs:rs+128); zero its P block there
                                nc.gpsimd.memset(pt[:, 1, :, rs:rs + 128], 0.0)
                        else:
                            pt = p8_pool.tile([128, 2, 2, 512], MM_DT,
                                              tag="p16", name="p16", bufs=2)
                        for par in range(2):
                            kt = kt0 + par
                            j = kt - 4 * qc
                            rss = 0 if j < 0 else j * 128
                            stp = ps_st.tile([128, 2, 512], F32,
                                             tag="stp", name="stp")
                            for hh in range(2):
                                nc.tensor.matmul(
                                    stp[:, hh, rss:512],
                                    kp[p][hh * HD:(hh + 1) * HD,
                                          kt * 128:(kt + 1) * 128],
                                    qp[p][hh * HD:(hh + 1) * HD,
                                          qc * 512 + rss:(qc + 1) * 512],
                                    start=True,
                                    stop=True,
                                    tile_position=(hh * HD, 0),
                                )
                            if j >= 0:
                                # additive triangular mask on the 128 cols
                                # that cross the diagonal
                                for hh in range(2):
                                    nc.vector.tensor_add(
                                        stp[:, hh, rss:rss + 128],
                                        stp[:, hh, rss:rss + 128],
                                        mask_sb[:, 128:256],
                                    )
                            # one EXP covers both heads of this key tile
                            nc.scalar.activation(
                                pt[:, par, :, rss:512],
                                stp[:, :, rss:512],
                                EXP,
                                bias=(shift_sb if fp8 else 0.0),
                            )
                        pend[ktp] = (pt, rs)
                    if ktp >= DEPTH:
                        ktp2 = ktp - DEPTH
                        pt, rs2 = pend.pop(ktp2)
                        if fp8:
                            for hh in range(2):
                                nc.tensor.matmul(
                                    at_ps[hh][:, rs2:512],
                                    v8[ktp2][:, :, 2 * p + hh, :],
                                    pt[:, :, hh, rs2:512],
                                    start=(ktp2 == 0),
                                    stop=(ktp2 == n_ktp - 1),
                                    perf_mode=DR,
                                )
                        else:
                            for par in range(2):
                                kt = 2 * ktp2 + par
                                j = kt - 4 * qc
                                rss = 0 if j < 0 else j * 128
                                for hh in range(2):
                                    nc.tensor.matmul(
                                        at_ps[hh][0:HD + 1, rss:512],
                                        v16[kt][:, 2 * p + hh, :],
                                        pt[:, par, hh, rss:512],
                                        start=(kt == 0),
                                        stop=(kt == 2 * n_ktp - 1),
                                    )
                    fill()
                # Release the accumulator banks fast (two DVE copies);
                # the slow reciprocal chain is deferred to emit_normalize
                # so it never sits ahead of the next segment's DVE work.
                for hh in range(2):
                    anu = an_pool.tile([HD, 512], MM_DT, tag="anu", name="anu")
                    nc.vector.tensor_copy(anu, at_ps[hh][0:HD, :])
                    lsb = sm_pool.tile([1, 512], F32, tag="lsb", name="lsb")
                    nc.vector.tensor_copy(lsb, at_ps[hh][HD:HD + 1, :])
                    pend_norm[(qc, p, hh)] = (anu, lsb)
                if last:
                    emit_normalize(qc, [p])
            while fillers:
                fillers.pop(0)()

        def emit_normalize(qc, pairs=range(NP)):
            an = an_hist[qc]
            for p in pairs:
                for hh in range(2):
                    anu, lsb = pend_norm.pop((qc, p, hh))
                    rec = sm_pool.tile([1, 512], F32, tag="rec", name="rec")
                    nc.vector.reciprocal_approx_fast(rec, lsb)
                    bc = sm_pool.tile([HD, 512], F32, tag="bc", name="bc")
                    nc.gpsimd.partition_broadcast(bc, rec)
                    nc.vector.tensor_mul(
                        an[p][hh * HD:(hh + 1) * HD, :], anu, bc,
                    )

        def outproj_groups(qc):
            an = an_hist.pop(qc)
            groups = []
            for qs in range(4):
                for e in range(2):
                    def run(qs=qs, e=e, an=an):
                        qsl = slice(qs * 128, (qs + 1) * 128)
                        ops = ps_mm.tile([128, 512], F32, tag="acc",
                                         name="psout")
                        for p in range(NP):
                            nc.tensor.matmul(
                                ops,
                                an[p][:, qsl],
                                owT_sb[p][:, e * 512:(e + 1) * 512],
                                start=(p == 0),
                                stop=(p == NP - 1),
                            )
                        osb = out_pool.tile([128, 512], MM_DT, tag="osb",
                                            name="osb")
                        nc.vector.tensor_copy(osb, ops)
                        nc.sync.dma_start(
                            out=out_d[qc * 512 + qs * 128:
                                      qc * 512 + (qs + 1) * 128,
                                      e * 512:(e + 1) * 512],
                            in_=osb,
                        )
                    groups.append(run)
            return groups

        # schedule: chunk order 0,2,3,1; projections and the previous
        # chunk's out-projection ride as fillers inside attention.
        emit_attention(0, [
            qk_group(2, 1), qk_group(3, 1), qk_group(2, 2), qk_group(3, 2),
            qk_group(0, 2), qk_group(1, 2),
            v_group(4), v_group(5), v_group(6), v_group(7),
            v_group(8), v_group(9), v_group(10), v_group(11),
        ])
        emit_normalize(0)
        emit_attention(2, [
            qk_group(2, 3), qk_group(3, 3), qk_group(0, 3), qk_group(1, 3),
            v_group(12), v_group(13), v_group(14), v_group(15),
        ] + outproj_groups(0))
        emit_normalize(2)
        emit_attention(3, [
            qk_group(0, 1), qk_group(1, 1),
        ] + outproj_groups(2))
        emit_normalize(3)
        emit_attention(1, outproj_groups(3), last=True)
        for g in outproj_groups(1):
            g()


_CACHE = {}


def _build():
    if "nc" in _CACHE:
        return _CACHE["nc"]
    nc = bacc.Bacc("TRN2", target_bir_lowering=False, debug=False)
    xT_d = nc.dram_tensor("xT", [D, S], MM_DT, kind="ExternalInput").ap()
    wT_d = nc.dram_tensor("wT", [D, 3 * VW], MM_DT, kind="ExternalInput").ap()
    owT_d = nc.dram_tensor("owT", [VW, D], MM_DT, kind="ExternalInput").ap()
    mask_d = nc.dram_tensor("mask", [128, 640], F32, kind="ExternalInput").ap()
    out_d = nc.dram_tensor("out", [S, D], MM_DT, kind="ExternalOutput").ap()
    with tile.TileContext(nc) as tc:
        _emit(tc, nc, xT_d, wT_d, owT_d, mask_d, out_d)
    nc.compile()
    _CACHE["nc"] = nc
    return nc


def _mask_np():
    # [128, 640]: cols 0-127 all NEG, cols 128-255 lower-triangular keep
    # (col >= row -> 0 else NEG), cols 256-639 zeros.
    m = np.zeros((128, 640), np.float32)
    m[:, 0:128] = NEG
    r = np.arange(128)
    tri = np.where(r[None, :] >= r[:, None], 0.0, NEG).astype(np.float32)
    m[:, 128:256] = tri
    return m


def make_in_maps(x, qkv_w, out_w):
    """Per-core input dicts for the 8-way (batch x head-group) sharding."""
    x = np.asarray(x, np.float32)
    qkv_w = np.asarray(qkv_w, np.float32)
    out_w = np.asarray(out_w, np.float32)
    xT = [np.ascontiguousarray(x[b].T) for b in range(B)]
    mask = _mask_np()
    import ml_dtypes
    np_mm = ml_dtypes.bfloat16
    in_maps = []
    for c in range(NCORES):
        b = c // 4
        h0 = (c % 4) * NH
        rows = np.arange(h0 * HD, (h0 + NH) * HD)
        wq = qkv_w[rows] * np.float32(SCALE)
        wk = qkv_w[D + rows]
        wv = qkv_w[2 * D + rows]
        wT = np.ascontiguousarray(np.concatenate([wq, wk, wv], 0).T)
        owT = np.ascontiguousarray(out_w[:, rows].T)
        in_maps.append({"xT": xT[b].astype(np_mm), "wT": wT.astype(np_mm),
                        "owT": owT.astype(np_mm), "mask": mask})
    return in_maps


def kernel(x, qkv_w, out_w, _trace=False, _trace_cores=None):
    nc = _build()
    in_maps = make_in_maps(x, qkv_w, out_w)
    res = run_bass_kernel_spmd(
        nc, in_maps, core_ids=list(range(NCORES)),
        trace=_trace, trace_cores=_trace_cores,
    )
    outs = [np.asarray(r["out"], dtype=np.float32) for r in res.results]
    full = np.stack([
        outs[0] + outs[1] + outs[2] + outs[3],
        outs[4] + outs[5] + outs[6] + outs[7],
    ]).astype(np.float32)
    if _trace:
        return full, res
    return full
